# revision 14
# baseline (speedup 1.0000x reference)
"""HGNN conv distributed Bass kernel for 8 TRN2 NeuronCores.

Computes  out = 0.5 * D_e ⊙ (MT.T @ (D_v ⊙ (MT @ (x @ W))))
with N=16384 nodes, E=8192 hyperedges, IN_FT=256, OUT_FT=128.

Sharding (node/data parallel per hint): MT columns, x rows are sharded
over nodes across the 8 cores; W is replicated. The MT @ y contraction
over nodes becomes a partial sum + AllReduce; the MT.T @ z contraction
over edges is local per node shard.

Host-side folds: sqrt(D_v) is folded into MT (so no edge scaling on
device), 0.5*D_e is applied on the host after gathering, x is sent
pre-transposed, and the device returns ny^T (host transposes back).

Per core the MT shard [E, N/8] streams through SBUF exactly once
(bf16, host-cast), fused over both phases in superblocks of EB=1024
edges:
  phase 1 needs MT.T tiles (contraction over nodes -> nodes on
  partitions), produced by PE transposes. eyT partials accumulate in
  PSUM, cast to bf16, and AllReduce'd across cores once per
  superblock — the 8 collectives pipeline against the phase-1
  cadence so only the last one's latency is exposed. The reduced z is
  produced by a DMA crossbar transpose straight out of the AllReduce
  DRAM buffer.
  phase 2 uses the natural MT tiles with the reduced z as stationary,
  accumulating ny^T in 4 persistent PSUM banks across all superblocks
  (no SBUF read-modify-write).
"""

import functools
from contextlib import ExitStack

import ml_dtypes
import numpy as np

import concourse.bass as bass
import concourse.mybir as mybir
import concourse.tile as tile
from concourse import bacc
from concourse.bass_utils import run_bass_kernel_spmd
from concourse.masks import make_identity

P = 128
BF16 = mybir.dt.bfloat16
F32 = mybir.dt.float32

FULL_CFG = dict(N=16384, E=8192, IN=256, F=128, CORES=8, G=8)


def build_kernel(nc, cfg):
    N, E, IN, F, CORES, G = (
        cfg["N"], cfg["E"], cfg["IN"], cfg["F"], cfg["CORES"], cfg["G"])
    NS = N // CORES          # nodes per core (2048)
    EB = E // G              # edges per superblock (1024)
    ET = EB // P             # 128-edge chunks per superblock (8)
    NJ = NS // P             # 128-node chunks (16)
    KI = IN // P             # 128-in_ft chunks (2)
    HALF = 512               # phase-1 edge half width (psum group)
    NH = EB // HALF          # halves per superblock (2)
    HT = HALF // P           # 128-edge chunks per half (4)
    NQ = NS // HALF          # 512-node windows (4)
    P2LAG = 2                # superblocks between AR issue and phase 2
    assert EB % P == 0 and NS % P == 0 and IN % P == 0 and F == P

    mt = nc.dram_tensor("mt", [E, NS], BF16, kind="ExternalInput").ap()
    xst = nc.dram_tensor("xst", [IN, NS], BF16, kind="ExternalInput").ap()
    w = nc.dram_tensor("w", [IN, F], BF16, kind="ExternalInput").ap()
    out = nc.dram_tensor("out", [F, NS], F32, kind="ExternalOutput").ap()

    with tile.TileContext(nc) as tc, ExitStack() as ctx:
        consts = ctx.enter_context(tc.tile_pool(name="consts", bufs=1))
        sbig = ctx.enter_context(tc.tile_pool(name="sbig", bufs=1))
        mtp = ctx.enter_context(tc.tile_pool(name="mtp", bufs=4))
        mtT_p = ctx.enter_context(tc.tile_pool(name="mtT", bufs=1))
        eyp_p = ctx.enter_context(tc.tile_pool(name="eyp", bufs=2))
        z_p = ctx.enter_context(tc.tile_pool(name="zp", bufs=3))
        ps_ey = ctx.enter_context(tc.tile_pool(name="ps_ey", bufs=1, space="PSUM"))
        ps_tr = ctx.enter_context(tc.tile_pool(name="ps_tr", bufs=2, space="PSUM"))
        ps_ny = ctx.enter_context(tc.tile_pool(name="ps_ny", bufs=1, space="PSUM"))
        dram = ctx.enter_context(tc.tile_pool(name="dram", bufs=2, space="DRAM"))

        # Small loads first so step A isn't stuck behind the 4MB MT block.
        id16 = consts.tile([P, P], BF16, tag="id16")
        make_identity(nc, id16[:])
        w_sb = consts.tile([P, KI, F], BF16, tag="w")
        nc.sync.dma_start(w_sb[:], w.rearrange("(k p) f -> p k f", p=P))
        xsT_sb = sbig.tile([P, KI, NS], BF16, tag="xsT")
        nc.scalar.dma_start(xsT_sb[:], xst.rearrange("(k p) n -> p k n", p=P))

        # MT superblock loads, split per edge half so phase 1 can start on
        # the first half while the second streams in.
        def load_mt(g):
            mt_sb = mtp.tile([P, ET, NS], BF16, tag="mt")
            for h in range(NH):
                t0, t1 = h * HT, (h + 1) * HT
                nc.sync.dma_start(
                    mt_sb[:, t0:t1, :],
                    mt[g * EB + t0 * P:g * EB + t1 * P, :].rearrange(
                        "(t p) n -> p t n", p=P),
                )
            return mt_sb

        mt_first = load_mt(0)

        yT_sb = sbig.tile([P, NS], BF16, tag="yT")
        y_sb = sbig.tile([P, NS], BF16, tag="y")
        ny_out = sbig.tile([P, NS], F32, tag="ny_out")

        # Persistent phase-2 accumulator: ny^T [F, NS] f32 (4 PSUM banks).
        # Also used as scratch for step A's y^T windows (phase 2's
        # start=True overwrites it afterwards).
        ny_ps = ps_ny.tile([P, NS], F32, tag="ny")

        # Copy-engine alternation between DVE and ACT to split the
        # PSUM->SBUF transpose-copy load across two engines.
        cp_state = [0]

        def copy_eng():
            cp_state[0] ^= 1
            if cp_state[0]:
                return nc.vector.tensor_copy
            return nc.scalar.copy

        # ---- Step A: y = xs @ w, via yT = w.T @ xsT ----------------------
        for q in range(NQ):
            for k in range(KI):
                nc.tensor.matmul(
                    ny_ps[:, q * HALF:(q + 1) * HALF],
                    lhsT=w_sb[:, k, :],
                    rhs=xsT_sb[:, k, q * HALF:(q + 1) * HALF],
                    start=(k == 0),
                    stop=(k == KI - 1),
                )
        nc.vector.tensor_copy(yT_sb[:], ny_ps[:])
        ytr = ps_ey.tile([P, NS], BF16, tag="ey")
        for i in range(NJ):
            nc.tensor.transpose(
                ytr[:, i * P:(i + 1) * P], yT_sb[:, i * P:(i + 1) * P], id16[:])
        nc.vector.tensor_copy(y_sb[:], ytr[:])

        # ---- Per-superblock phase 1 + AllReduce --------------------------
        def p1_superblock(g):
            mt_sb = mt_first if g == 0 else load_mt(g)
            # Transpose bursts: mtT[p, j, e] = MT^T[j*128+p, g*EB+e]
            mtT = mtT_p.tile([P, NJ, EB], BF16, tag="mtT")
            for h in range(NH):
                for jj in range(NJ // 2):
                    # One PSUM bank holds the transposes for two j-chunks.
                    tr = ps_tr.tile([P, 2 * HALF], BF16, tag="tr")
                    for c in range(2):
                        j = 2 * jj + c
                        for ti in range(HT):
                            t = h * HT + ti
                            nc.tensor.transpose(
                                tr[:, c * HALF + ti * P:c * HALF + (ti + 1) * P],
                                mt_sb[:, t, j * P:(j + 1) * P],
                                id16[:],
                            )
                    copy_eng()(
                        mtT[:, 2 * jj:2 * jj + 2, h * HALF:(h + 1) * HALF],
                        tr[:].rearrange("p (c e) -> p c e", c=2),
                    )
            # Matmul burst, j-major so consecutive matmuls share lhsT.
            eyT = ps_ey.tile([P, EB], F32, tag="ey")
            for j in range(NJ):
                for h in range(NH):
                    nc.tensor.matmul(
                        eyT[:, h * HALF:(h + 1) * HALF],
                        lhsT=y_sb[:, j * P:(j + 1) * P],
                        rhs=mtT[:, j, h * HALF:(h + 1) * HALF],
                        start=(j == 0),
                        stop=(j == NJ - 1),
                    )
            eyp = eyp_p.tile([P, EB], BF16, tag="eyp")
            nc.vector.tensor_copy(eyp[:], eyT[:])
            bin_t = dram.tile([P, EB], BF16, tag="bin")
            bout_t = dram.tile([P, EB], BF16, tag="bout", addr_space="Shared")
            nc.sync.dma_start(bin_t[:], eyp[:])
            nc.gpsimd.collective_compute(
                "AllReduce",
                mybir.AluOpType.add,
                replica_groups=[list(range(CORES))],
                ins=[bin_t.opt()],
                outs=[bout_t.opt()],
            )
            # z[p, t, f] = ey[t*128+p, f] for this superblock's 1024 edges,
            # transposed straight out of the AllReduce output in DRAM.
            z = z_p.tile([P, ET, P], BF16, tag="z")
            nc.scalar.dma_start_transpose(z[:, :, :], bout_t[:])
            return mt_sb, z

        def p2_superblock(g, mt_sb, z):
            for q in range(NQ):
                for t in range(ET):
                    nc.tensor.matmul(
                        ny_ps[:, q * HALF:(q + 1) * HALF],
                        lhsT=z[:, t, :],
                        rhs=mt_sb[:, t, q * HALF:(q + 1) * HALF],
                        start=(g == 0 and t == 0),
                        stop=(g == G - 1 and t == ET - 1),
                    )

        blocks = {}
        for g in range(G):
            blocks[g] = p1_superblock(g)
            if g >= P2LAG:
                p2_superblock(g - P2LAG, *blocks[g - P2LAG])
        for g in range(G - P2LAG, G):
            p2_superblock(g, *blocks[g])

        # ---- Finalize: out = ny^T (host applies 0.5*D_e and transposes) --
        nc.vector.tensor_copy(ny_out[:], ny_ps[:])
        nc.sync.dma_start(out, ny_out[:])

    return nc


@functools.lru_cache(maxsize=2)
def _compiled(cfg_items):
    cfg = dict(cfg_items)
    nc = bacc.Bacc(
        "TRN2",
        target_bir_lowering=False,
        debug=False,
        num_devices=cfg["CORES"],
    )
    build_kernel(nc, cfg)
    nc.compile()
    return nc


def shard_inputs(x, weight, MT, D_v_diag, D_e_diag, cfg):
    """Host-side sharding + dtype prep. Returns in_maps for the 8 cores."""
    N, E, IN, F, CORES = cfg["N"], cfg["E"], cfg["IN"], cfg["F"], cfg["CORES"]
    NS = N // CORES
    bf = ml_dtypes.bfloat16
    w_b = np.ascontiguousarray(np.asarray(weight, dtype=np.float32)).astype(bf)
    x_f = np.asarray(x, dtype=np.float32)
    dv = np.asarray(D_v_diag, dtype=np.float32)
    # fold sqrt(D_v) into MT rows: MT' = sqrt(dv) * MT, so that
    # MT'.T @ MT' = MT.T @ D_v @ MT and no edge scaling is needed on device
    mt_f = np.asarray(MT, dtype=np.float32) * np.sqrt(dv)[:, None]
    in_maps = []
    for c in range(CORES):
        sl = slice(c * NS, (c + 1) * NS)
        in_maps.append({
            "mt": np.ascontiguousarray(mt_f[:, sl]).astype(bf),
            "xst": np.ascontiguousarray(x_f[sl].T).astype(bf),
            "w": w_b,
        })
    return in_maps


def _run(x, weight, MT, D_v_diag, D_e_diag, cfg=None, trace=False):
    cfg = cfg or FULL_CFG
    nc = _compiled(tuple(sorted(cfg.items())))
    in_maps = shard_inputs(x, weight, MT, D_v_diag, D_e_diag, cfg)
    res = run_bass_kernel_spmd(
        nc, in_maps, core_ids=list(range(cfg["CORES"])), trace=trace)
    NS = cfg["N"] // cfg["CORES"]
    de = np.asarray(D_e_diag, dtype=np.float32)
    parts = []
    for c in range(cfg["CORES"]):
        nyT = np.asarray(res.results[c]["out"]).astype(np.float32)
        sl = slice(c * NS, (c + 1) * NS)
        parts.append((0.5 * de[sl])[:, None] * nyT.T)
    out = np.concatenate(parts, axis=0)
    return out, res


def kernel(x, weight, MT, D_v_diag, D_e_diag):
    out, _ = _run(x, weight, MT, D_v_diag, D_e_diag)
    return out


# revision 18
# speedup vs baseline: 1.0684x; 1.0684x over previous
"""HGNN conv distributed Bass kernel for 8 TRN2 NeuronCores.

Computes  out = 0.5 * D_e ⊙ (MT.T @ (D_v ⊙ (MT @ (x @ W))))
with N=16384 nodes, E=8192 hyperedges, IN_FT=256, OUT_FT=128.

Sharding (node/data parallel per hint): MT columns, x rows are sharded
over nodes across the 8 cores; W is replicated. The MT @ y contraction
over nodes becomes a partial sum + AllReduce; the MT.T @ z contraction
over edges is local per node shard.

Host-side folds: sqrt(D_v) is folded into MT (so no edge scaling on
device), 0.5*D_e is applied on the host after gathering, x is sent
pre-transposed, and the device returns ny^T (host transposes back).

Per core the MT shard [E, N/8] streams through SBUF exactly once
(bf16, host-cast), fused over both phases in superblocks of EB=1024
edges:
  phase 1 needs MT.T tiles (contraction over nodes -> nodes on
  partitions), produced by PE transposes. eyT partials accumulate in
  PSUM, cast to bf16, and AllReduce'd across cores once per
  superblock — the 8 collectives pipeline against the phase-1
  cadence so only the last one's latency is exposed. The reduced z is
  produced by a DMA crossbar transpose straight out of the AllReduce
  DRAM buffer.
  phase 2 uses the natural MT tiles with the reduced z as stationary,
  accumulating ny^T in 4 persistent PSUM banks across all superblocks
  (no SBUF read-modify-write).
"""

import functools
from contextlib import ExitStack

import ml_dtypes
import numpy as np

import concourse.bass as bass
import concourse.mybir as mybir
import concourse.tile as tile
from concourse import bacc
from concourse.bass_utils import run_bass_kernel_spmd
from concourse.masks import make_identity

P = 128
BF16 = mybir.dt.bfloat16
F32 = mybir.dt.float32

FULL_CFG = dict(N=16384, E=8192, IN=256, F=128, CORES=8, G=8)


def build_kernel(nc, cfg):
    N, E, IN, F, CORES, G = (
        cfg["N"], cfg["E"], cfg["IN"], cfg["F"], cfg["CORES"], cfg["G"])
    NS = N // CORES          # nodes per core (2048)
    EB = E // G              # edges per superblock (1024)
    ET = EB // P             # 128-edge chunks per superblock (8)
    NJ = NS // P             # 128-node chunks (16)
    KI = IN // P             # 128-in_ft chunks (2)
    HALF = 512               # phase-1 edge half width (psum group)
    NH = EB // HALF          # halves per superblock (2)
    HT = HALF // P           # 128-edge chunks per half (4)
    NQ = NS // HALF          # 512-node windows (4)
    P2LAG = 2                # superblocks between AR issue and phase 2
    assert EB % P == 0 and NS % P == 0 and IN % P == 0 and F == P

    mt = nc.dram_tensor("mt", [E, NS], BF16, kind="ExternalInput").ap()
    xst = nc.dram_tensor("xst", [IN, NS], BF16, kind="ExternalInput").ap()
    w = nc.dram_tensor("w", [IN, F], BF16, kind="ExternalInput").ap()
    out = nc.dram_tensor("out", [F, NS], F32, kind="ExternalOutput").ap()

    with tile.TileContext(nc) as tc, ExitStack() as ctx:
        consts = ctx.enter_context(tc.tile_pool(name="consts", bufs=1))
        sbig = ctx.enter_context(tc.tile_pool(name="sbig", bufs=1))
        mtp = ctx.enter_context(tc.tile_pool(name="mtp", bufs=4))
        mtT_p = ctx.enter_context(tc.tile_pool(name="mtT", bufs=1))
        eyp_p = ctx.enter_context(tc.tile_pool(name="eyp", bufs=2))
        z_p = ctx.enter_context(tc.tile_pool(name="zp", bufs=3))
        ps_ey = ctx.enter_context(tc.tile_pool(name="ps_ey", bufs=1, space="PSUM"))
        ps_tr = ctx.enter_context(tc.tile_pool(name="ps_tr", bufs=2, space="PSUM"))
        ps_ny = ctx.enter_context(tc.tile_pool(name="ps_ny", bufs=1, space="PSUM"))
        dram = ctx.enter_context(tc.tile_pool(name="dram", bufs=2, space="DRAM"))

        # Small loads first so step A isn't stuck behind the 4MB MT block.
        id16 = consts.tile([P, P], BF16, tag="id16")
        make_identity(nc, id16[:])
        w_sb = consts.tile([P, KI, F], BF16, tag="w")
        nc.sync.dma_start(w_sb[:], w.rearrange("(k p) f -> p k f", p=P))
        xsT_sb = sbig.tile([P, KI, NS], BF16, tag="xsT")
        nc.scalar.dma_start(xsT_sb[:], xst.rearrange("(k p) n -> p k n", p=P))

        # MT superblock loads, split per edge half so phase 1 can start on
        # the first half while the second streams in.
        def load_mt(g):
            mt_sb = mtp.tile([P, ET, NS], BF16, tag="mt")
            for h in range(NH):
                t0, t1 = h * HT, (h + 1) * HT
                nc.sync.dma_start(
                    mt_sb[:, t0:t1, :],
                    mt[g * EB + t0 * P:g * EB + t1 * P, :].rearrange(
                        "(t p) n -> p t n", p=P),
                )
            return mt_sb

        mt_first = load_mt(0)

        yT_sb = sbig.tile([P, NS], BF16, tag="yT")
        y_sb = sbig.tile([P, NS], BF16, tag="y")
        ny_out = sbig.tile([P, NS], F32, tag="ny_out")

        # Persistent phase-2 accumulator: ny^T [F, NS] f32 (4 PSUM banks).
        # Also used as scratch for step A's y^T windows (phase 2's
        # start=True overwrites it afterwards).
        ny_ps = ps_ny.tile([P, NS], F32, tag="ny")

        # Copy-engine alternation between DVE and ACT to split the
        # PSUM->SBUF transpose-copy load across two engines.
        cp_state = [0]

        def copy_eng():
            cp_state[0] ^= 1
            if cp_state[0]:
                return nc.vector.tensor_copy
            return nc.scalar.copy

        # ---- Step A: y = xs @ w, via yT = w.T @ xsT ----------------------
        for q in range(NQ):
            for k in range(KI):
                nc.tensor.matmul(
                    ny_ps[:, q * HALF:(q + 1) * HALF],
                    lhsT=w_sb[:, k, :],
                    rhs=xsT_sb[:, k, q * HALF:(q + 1) * HALF],
                    start=(k == 0),
                    stop=(k == KI - 1),
                )
        nc.vector.tensor_copy(yT_sb[:], ny_ps[:])
        ytr = ps_ey.tile([P, NS], BF16, tag="ey")
        for i in range(NJ):
            nc.tensor.transpose(
                ytr[:, i * P:(i + 1) * P], yT_sb[:, i * P:(i + 1) * P], id16[:])
        nc.vector.tensor_copy(y_sb[:], ytr[:])

        # ---- Phase 1 / AllReduce groups / phase 2 ------------------------
        # AllReduce groups of superblocks: the first is small so the
        # collective chain starts early, the last is small so the exposed
        # tail after the final p1 superblock is short.
        groups = [[0], [1, 2], [3, 4], [5, 6], [7]]
        sb_group = {}
        for gi, sbs in enumerate(groups):
            for off, g in enumerate(sbs):
                sb_group[g] = (gi, off)

        mt_sbs = {}
        eyps = {}
        zs = {}

        def p1_superblock(g):
            mt_sb = mt_first if g == 0 else load_mt(g)
            mt_sbs[g] = mt_sb
            # Transpose bursts: mtT[p, j, e] = MT^T[j*128+p, g*EB+e]
            mtT = mtT_p.tile([P, NJ, EB], BF16, tag="mtT")
            for h in range(NH):
                for jj in range(NJ // 2):
                    # One PSUM bank holds the transposes for two j-chunks.
                    tr = ps_tr.tile([P, 2 * HALF], BF16, tag="tr")
                    for c in range(2):
                        j = 2 * jj + c
                        for ti in range(HT):
                            t = h * HT + ti
                            nc.tensor.transpose(
                                tr[:, c * HALF + ti * P:c * HALF + (ti + 1) * P],
                                mt_sb[:, t, j * P:(j + 1) * P],
                                id16[:],
                            )
                    copy_eng()(
                        mtT[:, 2 * jj:2 * jj + 2, h * HALF:(h + 1) * HALF],
                        tr[:].rearrange("p (c e) -> p c e", c=2),
                    )
            # Matmul bursts per edge half.
            eyT = ps_ey.tile([P, EB], F32, tag="ey")
            for h in range(NH):
                for j in range(NJ):
                    nc.tensor.matmul(
                        eyT[:, h * HALF:(h + 1) * HALF],
                        lhsT=y_sb[:, j * P:(j + 1) * P],
                        rhs=mtT[:, j, h * HALF:(h + 1) * HALF],
                        start=(j == 0),
                        stop=(j == NJ - 1),
                    )
            gi, off = sb_group[g]
            if off == 0:
                eyps[gi] = eyp_p.tile(
                    [P, len(groups[gi]) * EB], BF16, tag="eyp",
                    name="eyp")
            nc.vector.tensor_copy(
                eyps[gi][:, off * EB:(off + 1) * EB], eyT[:])

        def emit_ar(gi):
            n_sb = len(groups[gi])
            eyp = eyps[gi]
            bin_t = dram.tile([P, n_sb * EB], BF16, tag="bin")
            bout_t = dram.tile(
                [P, n_sb * EB], BF16, tag="bout", addr_space="Shared")
            nc.scalar.dma_start(bin_t[:], eyp[:])
            nc.gpsimd.collective_compute(
                "AllReduce",
                mybir.AluOpType.add,
                replica_groups=[list(range(CORES))],
                ins=[bin_t.opt()],
                outs=[bout_t.opt()],
            )
            # z[p, s, f] = ey[s*128+p, f] for the group's edges, transposed
            # straight out of the AllReduce output in DRAM.
            z = z_p.tile([P, n_sb * ET, P], BF16, tag="z", name="z")
            nc.scalar.dma_start_transpose(z[:, :, :], bout_t[:])
            zs[gi] = z

        def p2_superblock(g):
            gi, off = sb_group[g]
            mt_sb, z = mt_sbs[g], zs[gi]
            for q in range(NQ):
                for t in range(ET):
                    nc.tensor.matmul(
                        ny_ps[:, q * HALF:(q + 1) * HALF],
                        lhsT=z[:, off * ET + t, :],
                        rhs=mt_sb[:, t, q * HALF:(q + 1) * HALF],
                        start=(g == 0 and t == 0),
                        stop=(g == G - 1 and t == ET - 1),
                    )

        # Emission order: p2(sb) is placed >= 2 superblocks after its
        # AllReduce is issued (head-of-line slack on the in-order PE queue),
        # and early enough to release MT buffers for the bufs=4 pool.
        seq = [("p1", 0), ("ar", 0), ("p1", 1), ("p1", 2), ("ar", 1),
               ("p1", 3), ("p2", 0), ("p1", 4), ("ar", 2), ("p2", 1),
               ("p1", 5), ("p2", 2), ("p2", 3), ("p1", 6), ("ar", 3),
               ("p2", 4), ("p1", 7), ("ar", 4), ("p2", 5), ("p2", 6),
               ("p2", 7)]
        for kind, idx in seq:
            if kind == "p1":
                p1_superblock(idx)
            elif kind == "ar":
                emit_ar(idx)
            else:
                p2_superblock(idx)

        # ---- Finalize: out = ny^T (host applies 0.5*D_e and transposes) --
        nc.vector.tensor_copy(ny_out[:], ny_ps[:])
        nc.sync.dma_start(out, ny_out[:])

    return nc


@functools.lru_cache(maxsize=2)
def _compiled(cfg_items):
    cfg = dict(cfg_items)
    nc = bacc.Bacc(
        "TRN2",
        target_bir_lowering=False,
        debug=False,
        num_devices=cfg["CORES"],
    )
    build_kernel(nc, cfg)
    nc.compile()
    return nc


def shard_inputs(x, weight, MT, D_v_diag, D_e_diag, cfg):
    """Host-side sharding + dtype prep. Returns in_maps for the 8 cores."""
    N, E, IN, F, CORES = cfg["N"], cfg["E"], cfg["IN"], cfg["F"], cfg["CORES"]
    NS = N // CORES
    bf = ml_dtypes.bfloat16
    w_b = np.ascontiguousarray(np.asarray(weight, dtype=np.float32)).astype(bf)
    x_f = np.asarray(x, dtype=np.float32)
    dv = np.asarray(D_v_diag, dtype=np.float32)
    # fold sqrt(D_v) into MT rows: MT' = sqrt(dv) * MT, so that
    # MT'.T @ MT' = MT.T @ D_v @ MT and no edge scaling is needed on device
    mt_f = np.asarray(MT, dtype=np.float32) * np.sqrt(dv)[:, None]
    in_maps = []
    for c in range(CORES):
        sl = slice(c * NS, (c + 1) * NS)
        in_maps.append({
            "mt": np.ascontiguousarray(mt_f[:, sl]).astype(bf),
            "xst": np.ascontiguousarray(x_f[sl].T).astype(bf),
            "w": w_b,
        })
    return in_maps


def _run(x, weight, MT, D_v_diag, D_e_diag, cfg=None, trace=False):
    cfg = cfg or FULL_CFG
    nc = _compiled(tuple(sorted(cfg.items())))
    in_maps = shard_inputs(x, weight, MT, D_v_diag, D_e_diag, cfg)
    res = run_bass_kernel_spmd(
        nc, in_maps, core_ids=list(range(cfg["CORES"])), trace=trace)
    NS = cfg["N"] // cfg["CORES"]
    de = np.asarray(D_e_diag, dtype=np.float32)
    parts = []
    for c in range(cfg["CORES"]):
        nyT = np.asarray(res.results[c]["out"]).astype(np.float32)
        sl = slice(c * NS, (c + 1) * NS)
        parts.append((0.5 * de[sl])[:, None] * nyT.T)
    out = np.concatenate(parts, axis=0)
    return out, res


def kernel(x, weight, MT, D_v_diag, D_e_diag):
    out, _ = _run(x, weight, MT, D_v_diag, D_e_diag)
    return out


# revision 21
# speedup vs baseline: 1.1488x; 1.0752x over previous
"""HGNN conv distributed Bass kernel for 8 TRN2 NeuronCores.

Computes  out = 0.5 * D_e ⊙ (MT.T @ (D_v ⊙ (MT @ (x @ W))))
with N=16384 nodes, E=8192 hyperedges, IN_FT=256, OUT_FT=128.

Sharding (node/data parallel per hint): MT columns, x rows are sharded
over nodes across the 8 cores; W is replicated. The MT @ y contraction
over nodes becomes a partial sum + AllReduce; the MT.T @ z contraction
over edges is local per node shard.

Host-side folds: sqrt(D_v) is folded into MT (so no edge scaling on
device), 0.5*D_e is applied on the host after gathering, x is sent
pre-transposed, and the device returns ny^T (host transposes back).

Per core the MT shard [E, N/8] streams through SBUF exactly once
(bf16, host-cast), fused over both phases in superblocks of EB=1024
edges:
  phase 1 needs MT.T tiles (contraction over nodes -> nodes on
  partitions), produced by PE transposes. eyT partials accumulate in
  PSUM, cast to bf16, and AllReduce'd across cores once per
  superblock — the 8 collectives pipeline against the phase-1
  cadence so only the last one's latency is exposed. The reduced z is
  produced by a DMA crossbar transpose straight out of the AllReduce
  DRAM buffer.
  phase 2 uses the natural MT tiles with the reduced z as stationary,
  accumulating ny^T in 4 persistent PSUM banks across all superblocks
  (no SBUF read-modify-write).
"""

import functools
from contextlib import ExitStack

import ml_dtypes
import numpy as np

import concourse.bass as bass
import concourse.mybir as mybir
import concourse.tile as tile
from concourse import bacc
from concourse.bass_utils import run_bass_kernel_spmd
from concourse.masks import make_identity

P = 128
BF16 = mybir.dt.bfloat16
F32 = mybir.dt.float32

FULL_CFG = dict(N=16384, E=8192, IN=256, F=128, CORES=8, G=8)


def build_kernel(nc, cfg):
    N, E, IN, F, CORES, G = (
        cfg["N"], cfg["E"], cfg["IN"], cfg["F"], cfg["CORES"], cfg["G"])
    NS = N // CORES          # nodes per core (2048)
    EB = E // G              # edges per superblock (1024)
    ET = EB // P             # 128-edge chunks per superblock (8)
    NJ = NS // P             # 128-node chunks (16)
    KI = IN // P             # 128-in_ft chunks (2)
    HALF = 512               # phase-1 edge half width (psum group)
    NH = EB // HALF          # halves per superblock (2)
    HT = HALF // P           # 128-edge chunks per half (4)
    NQ = NS // HALF          # 512-node windows (4)
    P2LAG = 2                # superblocks between AR issue and phase 2
    assert EB % P == 0 and NS % P == 0 and IN % P == 0 and F == P

    mt = nc.dram_tensor("mt", [E, NS], BF16, kind="ExternalInput").ap()
    xst = nc.dram_tensor("xst", [IN, NS], BF16, kind="ExternalInput").ap()
    w = nc.dram_tensor("w", [IN, F], BF16, kind="ExternalInput").ap()
    out = nc.dram_tensor("out", [F, NS], F32, kind="ExternalOutput").ap()

    with tile.TileContext(nc) as tc, ExitStack() as ctx:
        consts = ctx.enter_context(tc.tile_pool(name="consts", bufs=1))
        sbig = ctx.enter_context(tc.tile_pool(name="sbig", bufs=1))
        mtp = ctx.enter_context(tc.tile_pool(name="mtp", bufs=4))
        mtT_p = ctx.enter_context(tc.tile_pool(name="mtT", bufs=1))
        eyp_p = ctx.enter_context(tc.tile_pool(name="eyp", bufs=2))
        eyf_p = ctx.enter_context(tc.tile_pool(name="eyf", bufs=1))
        z_p = ctx.enter_context(tc.tile_pool(name="zp", bufs=3))
        ps_ey = ctx.enter_context(tc.tile_pool(name="ps_ey", bufs=1, space="PSUM"))
        ps_tr = ctx.enter_context(tc.tile_pool(name="ps_tr", bufs=2, space="PSUM"))
        ps_ny = ctx.enter_context(tc.tile_pool(name="ps_ny", bufs=1, space="PSUM"))
        dram = ctx.enter_context(tc.tile_pool(name="dram", bufs=2, space="DRAM"))

        # Small loads first so step A isn't stuck behind the 4MB MT block.
        id16 = consts.tile([P, P], BF16, tag="id16")
        make_identity(nc, id16[:])
        w_sb = consts.tile([P, KI, F], BF16, tag="w")
        nc.sync.dma_start(w_sb[:], w.rearrange("(k p) f -> p k f", p=P))
        xsT_sb = sbig.tile([P, KI, NS], BF16, tag="xsT")
        nc.scalar.dma_start(xsT_sb[:], xst.rearrange("(k p) n -> p k n", p=P))

        # MT superblock loads, split per edge half so phase 1 can start on
        # the first half while the second streams in.
        def load_mt(g):
            mt_sb = mtp.tile([P, ET, NS], BF16, tag="mt")
            for h in range(NH):
                t0, t1 = h * HT, (h + 1) * HT
                nc.sync.dma_start(
                    mt_sb[:, t0:t1, :],
                    mt[g * EB + t0 * P:g * EB + t1 * P, :].rearrange(
                        "(t p) n -> p t n", p=P),
                )
            return mt_sb

        mt_first = load_mt(0)

        yT_sb = sbig.tile([P, NS], BF16, tag="yT")
        y_sb = sbig.tile([P, NS], BF16, tag="y")
        ny_out = sbig.tile([P, NS], F32, tag="ny_out")

        # Persistent phase-2 accumulator: ny^T [F, NS] f32 (4 PSUM banks).
        # Also used as scratch for step A's y^T windows (phase 2's
        # start=True overwrites it afterwards).
        ny_ps = ps_ny.tile([P, NS], F32, tag="ny")

        # Copy-engine alternation between DVE and ACT to split the
        # PSUM->SBUF transpose-copy load across two engines.
        cp_state = [0]

        def copy_eng():
            cp_state[0] ^= 1
            if cp_state[0]:
                return nc.vector.tensor_copy
            return nc.scalar.copy

        # ---- Step A: y = xs @ w, via yT = w.T @ xsT ----------------------
        for q in range(NQ):
            for k in range(KI):
                nc.tensor.matmul(
                    ny_ps[:, q * HALF:(q + 1) * HALF],
                    lhsT=w_sb[:, k, :],
                    rhs=xsT_sb[:, k, q * HALF:(q + 1) * HALF],
                    start=(k == 0),
                    stop=(k == KI - 1),
                )
        nc.vector.tensor_copy(yT_sb[:], ny_ps[:])
        ytr = ps_ey.tile([P, NS], BF16, tag="ey")
        for i in range(NJ):
            nc.tensor.transpose(
                ytr[:, i * P:(i + 1) * P], yT_sb[:, i * P:(i + 1) * P], id16[:])
        nc.vector.tensor_copy(y_sb[:], ytr[:])

        # ---- Phase 1 / AllReduce groups / phase 2 ------------------------
        # AllReduce groups of superblocks: the first is small so the
        # collective chain starts early, the last is small so the exposed
        # tail after the final p1 superblock is short.
        groups = [[0], [1, 2], [3, 4], [5, 6], [7]]
        sb_group = {}
        for gi, sbs in enumerate(groups):
            for off, g in enumerate(sbs):
                sb_group[g] = (gi, off)

        mt_sbs = {}
        eyps = {}
        zs = {}

        def p1_superblock(g):
            mt_sb = mt_first if g == 0 else load_mt(g)
            mt_sbs[g] = mt_sb
            # Transpose bursts: mtT[p, j, e] = MT^T[j*128+p, g*EB+e]
            mtT = mtT_p.tile([P, NJ, EB], BF16, tag="mtT")
            for h in range(NH):
                for jj in range(NJ // 2):
                    # One PSUM bank holds the transposes for two j-chunks.
                    tr = ps_tr.tile([P, 2 * HALF], BF16, tag="tr")
                    for c in range(2):
                        j = 2 * jj + c
                        for ti in range(HT):
                            t = h * HT + ti
                            nc.tensor.transpose(
                                tr[:, c * HALF + ti * P:c * HALF + (ti + 1) * P],
                                mt_sb[:, t, j * P:(j + 1) * P],
                                id16[:],
                            )
                    copy_eng()(
                        mtT[:, 2 * jj:2 * jj + 2, h * HALF:(h + 1) * HALF],
                        tr[:].rearrange("p (c e) -> p c e", c=2),
                    )
            # Matmul bursts per edge half.
            eyT = ps_ey.tile([P, EB], F32, tag="ey")
            for h in range(NH):
                for j in range(NJ):
                    nc.tensor.matmul(
                        eyT[:, h * HALF:(h + 1) * HALF],
                        lhsT=y_sb[:, j * P:(j + 1) * P],
                        rhs=mtT[:, j, h * HALF:(h + 1) * HALF],
                        start=(j == 0),
                        stop=(j == NJ - 1),
                    )
            gi, off = sb_group[g]
            if off == 0:
                eyps[gi] = eyp_p.tile(
                    [P, len(groups[gi]) * ET, P], BF16, tag="eyp",
                    name="eyp")
            # Cast the psum partials to bf16, then transpose into z layout
            # BEFORE the AllReduce, so the collective's output needs no
            # transform on the latency-critical AR -> phase-2 edge.
            eyp = eyf_p.tile([P, EB], BF16, tag="eyf", name="eyf")
            nc.vector.tensor_copy(eyp[:], eyT[:])
            for c in range(2):
                tr = ps_tr.tile([P, 2 * HALF], BF16, tag="tr", name="tr")
                for ti in range(HT):
                    t = c * HT + ti
                    nc.tensor.transpose(
                        tr[:, ti * P:(ti + 1) * P],
                        eyp[:, t * P:(t + 1) * P],
                        id16[:],
                    )
                copy_eng()(
                    eyps[gi][:, off * ET + c * HT:off * ET + (c + 1) * HT, :],
                    tr[:, :HT * P].rearrange("p (c e) -> p c e", c=HT),
                )

        def emit_ar(gi):
            n_sb = len(groups[gi])
            bin_t = dram.tile([P, n_sb * ET, P], BF16, tag="bin")
            bout_t = dram.tile(
                [P, n_sb * ET, P], BF16, tag="bout", addr_space="Shared")
            nc.scalar.dma_start(bin_t[:], eyps[gi][:])
            nc.gpsimd.collective_compute(
                "AllReduce",
                mybir.AluOpType.add,
                replica_groups=[list(range(CORES))],
                ins=[bin_t.opt()],
                outs=[bout_t.opt()],
            )
            # The AllReduce output is already in z layout; a plain load on
            # the gpsimd queue (so no hwdge queue is head-of-line blocked
            # waiting on the collective).
            z = z_p.tile([P, n_sb * ET, P], BF16, tag="z", name="z")
            nc.gpsimd.dma_start(z[:], bout_t[:])
            zs[gi] = z

        def p2_superblock(g):
            gi, off = sb_group[g]
            mt_sb, z = mt_sbs[g], zs[gi]
            for q in range(NQ):
                for t in range(ET):
                    nc.tensor.matmul(
                        ny_ps[:, q * HALF:(q + 1) * HALF],
                        lhsT=z[:, off * ET + t, :],
                        rhs=mt_sb[:, t, q * HALF:(q + 1) * HALF],
                        start=(g == 0 and t == 0),
                        stop=(g == G - 1 and t == ET - 1),
                    )

        # Emission order: p2(sb) is placed >= 2 superblocks after its
        # AllReduce is issued (head-of-line slack on the in-order PE queue),
        # and early enough to release MT buffers for the bufs=4 pool.
        seq = [("p1", 0), ("ar", 0), ("p1", 1), ("p1", 2), ("ar", 1),
               ("p1", 3), ("p2", 0), ("p1", 4), ("ar", 2), ("p2", 1),
               ("p1", 5), ("p2", 2), ("p2", 3), ("p1", 6), ("ar", 3),
               ("p2", 4), ("p1", 7), ("ar", 4), ("p2", 5), ("p2", 6),
               ("p2", 7)]
        for kind, idx in seq:
            if kind == "p1":
                p1_superblock(idx)
            elif kind == "ar":
                emit_ar(idx)
            else:
                p2_superblock(idx)

        # ---- Finalize: out = ny^T (host applies 0.5*D_e and transposes) --
        nc.vector.tensor_copy(ny_out[:], ny_ps[:])
        nc.sync.dma_start(out, ny_out[:])

    return nc


@functools.lru_cache(maxsize=2)
def _compiled(cfg_items):
    cfg = dict(cfg_items)
    nc = bacc.Bacc(
        "TRN2",
        target_bir_lowering=False,
        debug=False,
        num_devices=cfg["CORES"],
    )
    build_kernel(nc, cfg)
    nc.compile()
    return nc


def shard_inputs(x, weight, MT, D_v_diag, D_e_diag, cfg):
    """Host-side sharding + dtype prep. Returns in_maps for the 8 cores."""
    N, E, IN, F, CORES = cfg["N"], cfg["E"], cfg["IN"], cfg["F"], cfg["CORES"]
    NS = N // CORES
    bf = ml_dtypes.bfloat16
    w_b = np.ascontiguousarray(np.asarray(weight, dtype=np.float32)).astype(bf)
    x_f = np.asarray(x, dtype=np.float32)
    dv = np.asarray(D_v_diag, dtype=np.float32)
    # fold sqrt(D_v) into MT rows: MT' = sqrt(dv) * MT, so that
    # MT'.T @ MT' = MT.T @ D_v @ MT and no edge scaling is needed on device
    mt_f = np.asarray(MT, dtype=np.float32) * np.sqrt(dv)[:, None]
    in_maps = []
    for c in range(CORES):
        sl = slice(c * NS, (c + 1) * NS)
        in_maps.append({
            "mt": np.ascontiguousarray(mt_f[:, sl]).astype(bf),
            "xst": np.ascontiguousarray(x_f[sl].T).astype(bf),
            "w": w_b,
        })
    return in_maps


def _run(x, weight, MT, D_v_diag, D_e_diag, cfg=None, trace=False):
    cfg = cfg or FULL_CFG
    nc = _compiled(tuple(sorted(cfg.items())))
    in_maps = shard_inputs(x, weight, MT, D_v_diag, D_e_diag, cfg)
    res = run_bass_kernel_spmd(
        nc, in_maps, core_ids=list(range(cfg["CORES"])), trace=trace)
    NS = cfg["N"] // cfg["CORES"]
    de = np.asarray(D_e_diag, dtype=np.float32)
    parts = []
    for c in range(cfg["CORES"]):
        nyT = np.asarray(res.results[c]["out"]).astype(np.float32)
        sl = slice(c * NS, (c + 1) * NS)
        parts.append((0.5 * de[sl])[:, None] * nyT.T)
    out = np.concatenate(parts, axis=0)
    return out, res


def kernel(x, weight, MT, D_v_diag, D_e_diag):
    out, _ = _run(x, weight, MT, D_v_diag, D_e_diag)
    return out
